# revision 38
# baseline (speedup 1.0000x reference)
"""AverageSpanExtractor Trainium2 kernel — sorted-span windowed banded matmul.

Math: out[n, :] = mean(seq[start_n:end_n, :]) * mask_n
    = (1/width_n) * sum_s ind(start_n <= s < end_n) * seq[s, :]

Strategy (per core; data-parallel over batch across 8 cores):
  1. HOST: sort spans by start (pure permutation; output is un-permuted
     after download).  Tile t = sorted spans [128t, 128t+128).  Because
     span width <= 20 and starts are sorted, tile t's spans live in a
     ~270-token window -> only L_t ~ 3-4 of the 16 token blocks, instead
     of all 16.  Windows are unioned across the 8 cores so one SPMD
     program serves all of them; out-of-window mask entries are exactly 0
     so correctness never depends on the window choice being tight.
  2. Stream seq [S=2048, D=512] f16 into SBUF (triples, single blocks at
     the end so the trailing matmuls are not gated on one big chunk).
  3. Per token block b, the spans that can touch it are a contiguous
     ~100-column range of the sorted order (host-computed, unioned across
     cores).  The mask buffer for all 16 blocks is zeroed ONCE by an
     early DVE memset (in the dead time while the gating DMAs land), so
     per block only two narrow f16 ops run: Pool builds c = (end > s),
     DVE fuses m = (start <= s) * c via scalar_tensor_tensor
     (s = 128b + p; exact small-int compares in f16).
  4. pout_t[p, d] = sum_{b in window_t} m_b[:, 128t:].T @ x_b — ~30
     accumulating fp16 matmuls into 8 PSUM banks (vs 128 dense); MM issue
     rate is 215 ns at full clock.  Dep-free warmup matmuls keep the PE's
     HAM clock gate open until the first real matmul (idle >2us drops the
     PE to 1.2 GHz and it takes ~5us to climb back).
  5. As soon as a tile's window closes, scale by span_mask/width on ACT
     (tile 6 on DVE to decouple the back-to-back closures at the end)
     and store its contiguous 128 rows — stores overlap remaining blocks.
HBM traffic ~ 2.5 MiB in + 1 MiB out.
"""

import numpy as np

import concourse.bacc as bacc
import concourse.tile as tile
from concourse import mybir
from concourse.bass import AP

# Problem shape (hardcoded per contract).
B, S, D, N = 8, 2048, 512, 1024
W = 20                   # max span width (reference MAX_SPAN_WIDTH)
NBLK = S // 128          # 16 token blocks
NTILE = N // 128         # 8 span tiles
MW = 256                 # mask buffer columns per block (<= 2 tiles cover)

F32 = mybir.dt.float32
I32 = mybir.dt.int32
F16 = mybir.dt.float16

WARM_NARROW = 20         # dep-free PE warmup matmuls (un-throttle HAM clock)
WARM_WIDE = 2

# seq chunking: fine-grained so each block's semaphore fires as soon as
# its bytes land — the PE rides directly on the ~270 GB/s DMA feed
CHUNKS = [(0, 1), (1, 2), (2, 4), (4, 6), (6, 8), (8, 10), (10, 11),
          (11, 12), (12, 13), (13, 14), (14, 15), (15, 16)]


def build_kernel_body(tc: tile.TileContext, seq: AP, stb: AP, enb: AP,
                      meta: AP, out: AP, lo, hi, rng, ctx):
    nc = tc.nc
    sbuf = ctx.enter_context(tc.tile_pool(name="sbuf", bufs=1))
    const = ctx.enter_context(tc.tile_pool(name="const", bufs=1))
    mpool = ctx.enter_context(tc.tile_pool(name="mpool", bufs=4))
    opool = ctx.enter_context(tc.tile_pool(name="opool", bufs=1))
    psum = ctx.enter_context(tc.tile_pool(name="psum", bufs=1, space="PSUM"))

    pouts = [psum.tile([128, D], F32, name=f"pout{j}", tag=f"pout{j}")
             for j in range(NTILE)]

    # one persistent mask buffer for all 16 blocks, zeroed once (split
    # DVE/Pool) in the dead window before stb/enb land; the per-block STT
    # then fills only the active columns.
    mask_all = sbuf.tile([128, NBLK, MW], F16, tag="mask_all")
    nc.vector.memset(mask_all[:, 0:NBLK // 2, :], 0.0)
    nc.gpsimd.memset(mask_all[:, NBLK // 2:, :], 0.0)

    # PE warmup (borrows pout0; the real start=True accumulation clears it)
    wconst = const.tile([128, D], F16, tag="wconst")
    nc.gpsimd.memset(wconst[:], 0.0)
    for _ in range(WARM_NARROW):
        nc.tensor.matmul(out=pouts[0][:, 0:128], lhsT=wconst[:, 0:128],
                         rhs=wconst[:, 0:128], start=True, stop=True)
    for _ in range(WARM_WIDE):
        nc.tensor.matmul(out=pouts[0][:], lhsT=wconst[:, 0:128],
                         rhs=wconst[:], start=True, stop=True)

    # ---------------- loads (all descriptor-light) ----------------
    # ALL input loads go on the sync queue in exact consumption order:
    # the DMA rings share one engine pool, so a second queue only
    # scrambles arrival order (measured: both split-queue variants lost
    # 5us).  stb/enb gate the mask pipeline and go first.
    st_bc = sbuf.tile([128, N], F16, tag="st_bc")
    en_bc = sbuf.tile([128, N], F16, tag="en_bc")
    nc.sync.dma_start(st_bc[:], stb)
    nc.sync.dma_start(en_bc[:], enb)

    # seq arrives host-pre-shuffled as [128, blk*d (+pad)] partition-major,
    # so each chunk is ONE contiguous multi-KiB descriptor per partition;
    # the 64-element row pad breaks the power-of-two partition stride so
    # concurrent descriptors spread across HBM banks.
    xf = sbuf.tile([128, NBLK, D], F16, tag="xf")
    for q0, q1 in CHUNKS:
        sl = (slice(None), slice(q0, q1), slice(None))
        nc.sync.dma_start(
            xf[sl],
            seq[:, D * q0:D * q1].rearrange("p (j d) -> p j d", d=D))

    # meta[p, 0:8]=start, [8:16]=end, [16:24]=span_mask for sorted span
    # n = 128t + p at column t (pure host layout staging of the int inputs).
    meta_sb = sbuf.tile([128, 3 * NTILE], I32, tag="meta_sb")
    nc.gpsimd.dma_start(meta_sb[:], meta)

    # scale[p, t] = span_mask / width for sorted span 128t + p; Pool-heavy
    # and emitted up front so neither the DVE mask chain nor drains wait.
    w_i = sbuf.tile([128, NTILE], I32, tag="w_i")
    nc.gpsimd.tensor_tensor(out=w_i[:], in0=meta_sb[:, NTILE:2 * NTILE],
                            in1=meta_sb[:, 0:NTILE],
                            op=mybir.AluOpType.subtract)
    w_f = sbuf.tile([128, NTILE], F32, tag="w_f")
    nc.gpsimd.tensor_copy(w_f[:], w_i[:])
    r_f = sbuf.tile([128, NTILE], F32, tag="r_f")
    nc.vector.reciprocal(r_f[:], w_f[:])
    m_f = sbuf.tile([128, NTILE], F32, tag="m_f")
    nc.gpsimd.tensor_copy(m_f[:], meta_sb[:, 2 * NTILE:3 * NTILE])
    scale = sbuf.tile([128, NTILE], F32, tag="scale")
    nc.gpsimd.tensor_tensor(out=scale[:], in0=r_f[:], in1=m_f[:],
                            op=mybir.AluOpType.mult)

    tiles_of_b = [[t for t in range(NTILE) if lo[t] <= b <= hi[t]]
                  for b in range(NBLK)]

    # -------- masks (Pool compare + narrow DVE STT) + windowed matmuls ----
    for b in range(NBLK):
        ts = tiles_of_b[b]
        if not ts:
            continue
        tmin, tmax = ts[0], ts[-1]
        c0 = 128 * tmin
        r0, r1 = rng[b]                 # active sorted-span columns
        c_t = mpool.tile([128, r1 - r0], F16, name="ct", tag="ct")
        nc.vector.tensor_scalar(out=c_t[:], in0=en_bc[:, r0:r1],
                                scalar1=float(128 * b), scalar2=None,
                                op0=mybir.AluOpType.is_gt)
        nc.vector.scalar_tensor_tensor(
            out=mask_all[:, b, r0 - c0:r1 - c0], in0=st_bc[:, r0:r1],
            scalar=float(128 * b), in1=c_t[:],
            op0=mybir.AluOpType.is_le, op1=mybir.AluOpType.mult)
        for t in ts:
            o0 = 128 * t - c0
            nc.tensor.matmul(out=pouts[t][:],
                             lhsT=mask_all[:, b, o0:o0 + 128],
                             rhs=xf[:, b, :],
                             start=(b == lo[t]), stop=(b == hi[t]))
        # drain any tile whose window just closed; stores overlap the
        # remaining blocks' matmuls.  The final tile is split into two
        # independent halves (ACT || DVE, separate tiles and stores) to
        # compress the last-matmul -> last-store tail.
        for t in range(NTILE):
            if hi[t] != b:
                continue
            o_t = opool.tile([128, D], F16, name=f"o{t}", tag=f"o{t}")
            if t == 6:
                nc.vector.tensor_scalar(out=o_t[:], in0=pouts[t][:],
                                        scalar1=scale[:, t:t + 1],
                                        scalar2=None,
                                        op0=mybir.AluOpType.mult)
            else:
                nc.scalar.mul(o_t[:], pouts[t][:], scale[:, t:t + 1])
            # t6's store rides gpsimd so sync is free the moment the final
            # (t7) drain completes
            eng = nc.gpsimd if t in (1, 3, 5, 6) else nc.sync
            eng.dma_start(out[128 * t:128 * (t + 1), :], o_t[:])


def build_nc(lo, hi, rng):
    nc = bacc.Bacc("TRN2", target_bir_lowering=False, debug=False)
    seq = nc.dram_tensor("seq", [128, NBLK * D + 64], F16,
                         kind="ExternalInput")
    stb = nc.dram_tensor("stb", [128, N], F16, kind="ExternalInput")
    enb = nc.dram_tensor("enb", [128, N], F16, kind="ExternalInput")
    meta = nc.dram_tensor("meta", [128, 3 * NTILE], I32, kind="ExternalInput")
    out = nc.dram_tensor("out", [N, D], F16, kind="ExternalOutput")
    from contextlib import ExitStack
    with tile.TileContext(nc) as tc:
        with ExitStack() as ctx:
            build_kernel_body(tc, seq.ap(), stb.ap(), enb.ap(), meta.ap(),
                              out.ap(), lo, hi, rng, ctx)
    nc.compile()
    return nc


_NC_CACHE = {}


def prep(sequence_tensor, span_indices, span_indices_mask):
    """Host-side staging: sort spans by start, build per-core input maps,
    the (cross-core) per-tile block windows and per-block active column
    ranges."""
    spans = np.asarray(span_indices).astype(np.int64)
    st = spans[..., 0].astype(np.int32)                  # [B, N]
    en = spans[..., 1].astype(np.int32)
    # reference truncates spans wider than W to their last W tokens
    st = np.maximum(st, en - W).astype(np.int32)
    mk = np.asarray(span_indices_mask).astype(np.int32)
    seq_f16 = np.ascontiguousarray(sequence_tensor, dtype=np.float16)

    orders = np.argsort(st, axis=1, kind="stable")       # [B, N]
    st_s = np.take_along_axis(st, orders, 1)
    en_s = np.take_along_axis(en, orders, 1)
    mk_s = np.take_along_axis(mk, orders, 1)

    # per-tile block windows, unioned across cores
    st_t = st_s.reshape(B, NTILE, 128)
    en_t = en_s.reshape(B, NTILE, 128)
    lo = tuple((st_t.min(axis=2) // 128).min(axis=0).tolist())
    hi = tuple(((en_t.max(axis=2) - 1) // 128).max(axis=0).tolist())

    # per-block active columns: spans with st in [128b - (W-1), 128b + 127]
    # (anything else has exactly-zero mask in block b), unioned across cores
    rng = []
    for b in range(NBLK):
        r0 = int(min(np.searchsorted(st_s[c], 128 * b - (W - 1), "left")
                     for c in range(B)))
        r1 = int(max(np.searchsorted(st_s[c], 128 * b + 127, "right")
                     for c in range(B)))
        rng.append((r0, max(r1, r0 + 1)))
    rng = tuple(rng)

    # the active range must sit inside the covering tiles' columns and the
    # MW-wide mask buffer (holds mathematically; cheap to assert)
    for b in range(NBLK):
        ts = [t for t in range(NTILE) if lo[t] <= b <= hi[t]]
        assert ts and 128 * ts[0] <= rng[b][0] and rng[b][1] <= 128 * (ts[-1] + 1)
        assert rng[b][1] - 128 * ts[0] <= MW

    prow = np.arange(128, dtype=np.int32)[:, None]
    in_maps = []
    for b in range(B):
        stb = (st_s[b][None, :] - prow).astype(np.float16)
        enb = (en_s[b][None, :] - prow).astype(np.float16)
        meta = np.concatenate([st_s[b].reshape(NTILE, 128).T,
                               en_s[b].reshape(NTILE, 128).T,
                               mk_s[b].reshape(NTILE, 128).T],
                              axis=1).astype(np.int32)
        seq_pjd = np.zeros((128, NBLK * D + 64), dtype=np.float16)
        seq_pjd[:, :NBLK * D] = seq_f16[b].reshape(
            NBLK, 128, D).transpose(1, 0, 2).reshape(128, NBLK * D)
        in_maps.append({
            "seq": seq_pjd,
            "stb": np.ascontiguousarray(stb),
            "enb": np.ascontiguousarray(enb),
            "meta": np.ascontiguousarray(meta),
        })
    return (lo, hi, rng), in_maps, orders


def kernel(sequence_tensor: np.ndarray, span_indices: np.ndarray,
           span_indices_mask: np.ndarray) -> np.ndarray:
    from concourse.bass_utils import run_bass_kernel_spmd

    key, in_maps, orders = prep(sequence_tensor, span_indices,
                                span_indices_mask)
    nc = _NC_CACHE.get(key)
    if nc is None:
        nc = _NC_CACHE[key] = build_nc(*key)

    res = run_bass_kernel_spmd(nc, in_maps, core_ids=list(range(B)))
    out = np.empty((B, N, D), dtype=np.float32)
    for b in range(B):
        out[b][orders[b]] = res.results[b]["out"].astype(np.float32)
    return out
